# revision 7
# baseline (speedup 1.0000x reference)
"""Trilinear grid-encoding lookup (embedding_lookup) on 8 trn2 NeuronCores.

Strategy:
  - Host: expand the 128^3 x 16 latent grid into an "oct" table where entry
    (x, y, z) holds all 8 corner feature rows of cell (x,y,z) contiguously
    (8*16 f32 = 512 B).  Shard points by their x-cell into 8 contiguous
    lx-ranges (16 planes each); each core gets its oct-table slab (~130 MB)
    plus its point list, counting-sorted into 8 two-plane "windows" so that
    every gather stays inside a 31752-row window (int16 index range).
  - Device (per core): Vector engine computes cell indices + fractions + the
    8 trilinear weights; indices take a DRAM round-trip to reach the
    wrap-16-replicated int16 layout dma_gather requires; dma_gather pulls one
    512-B oct entry per point (one descriptor per point); Vector engine
    multiplies by broadcast weights and reduces over the 8 corners.
  - Host: inverse-permute the 8 core outputs into the full [N, 16] result.
"""

import numpy as np

import concourse.bass as bass
import concourse.tile as tile
from concourse import bacc, mybir
from concourse.bass_utils import run_bass_kernel_spmd
from concourse.library_config import mlp

F32 = mybir.dt.float32
I16 = mybir.dt.int16
I32 = mybir.dt.int32

# Problem constants (hardcoded per harness contract).
R = 128                  # grid resolution
RI = 126                 # R - 2 (coordinate scale)
F = 16                   # feature dim
NCORES = 8

# Sharding constants.
CELLS = 126              # lx/ly/lz take values 0..125
PLANE = CELLS * CELLS    # oct entries per x-plane (15876)
BOUNDS = [0, 16, 32, 48, 64, 80, 96, 112, 126]   # lx plane ranges per core
MAXPLANES = 16
NWIN = 8                 # two-plane gather windows per core
WROWS = 2 * PLANE        # oct rows per window (31752 < 2^15)
SLAB_ROWS = MAXPLANES * PLANE
ELEM = 8 * F             # f32 per oct entry (512 B)

TBIG = 8192              # points per big tile
TSMALL = 2048            # points per small (tail) tile
SBIG = TBIG // 128       # slots per partition in a big tile


def build_body(tc, out_d, pts_d, slab_d, base_t, pool_d, tiles_w):
    """Per-core tile kernel.

    tiles_w: per-window tuple of tile sizes, e.g. (8192,)*4 + (2048,).
    pts/out hold sum(all tiles) points in window-bucketed order.
    base_t: [128, NWIN] f32 SBUF tile of window base planes (per core).
    """
    nc = tc.nc
    mult = mybir.AluOpType.mult
    add = mybir.AluOpType.add
    sub = mybir.AluOpType.subtract

    cwmax = max(sum(s) for s in tiles_w)

    with (
        tc.tile_pool(name="small", bufs=3) as pool_s,
        tc.tile_pool(name="w8p", bufs=12) as pool_w8,
        tc.tile_pool(name="off", bufs=4) as pool_off,
        tc.tile_pool(name="g", bufs=3) as pool_g,
        tc.tile_pool(name="idxs", bufs=2) as pool_i,
        tc.tile_pool(name="res", bufs=3) as pool_r,
    ):
        base = 0                    # running point offset in pts/out
        for w, sizes in enumerate(tiles_w):
            cw = sum(sizes)
            woff = base
            wslab = slab_d[w * WROWS:(w + 1) * WROWS, :]
            idx_dram = pool_d.tile([cwmax], I16, tag="idxd", name="idxd")[:cw]

            # Phase A per tile: weights + window-relative offsets -> DRAM.
            w8s, tile_offs = [], []
            toff = 0
            for t_sz in sizes:
                tslot = t_sz // 128
                ptsv = pts_d[woff + toff:woff + toff + t_sz, :].rearrange(
                    "(s p) c -> p s c", p=128)
                pts_sb = pool_s.tile([128, SBIG, 3], F32, tag="pts", name="pts")[:, :tslot, :]
                nc.sync.dma_start(pts_sb, ptsv)

                s = pool_s.tile([128, SBIG, 3], F32, tag="s", name="s")[:, :tslot, :]
                nc.vector.tensor_scalar_mul(s, pts_sb, float(RI))
                # f32->int cast rounding differs between sim (trunc) and HW
                # (round-to-nearest); fix up with a negative-fraction mask so
                # lf == floor(s) and frac in [0,1) under either mode.
                li = pool_s.tile([128, SBIG, 3], I32, tag="li", name="li")[:, :tslot, :]
                nc.vector.tensor_copy(li, s)
                lf = pool_s.tile([128, SBIG, 3], F32, tag="lf", name="lf")[:, :tslot, :]
                nc.vector.tensor_copy(lf, li)

                # wab[:, :, 0:3] = 1 - frac, wab[:, :, 3:6] = frac
                wab = pool_s.tile([128, SBIG, 6], F32, tag="wab", name="wab")[:, :tslot, :]
                nc.vector.tensor_tensor(wab[:, :, 3:6], s, lf, sub)
                mk = pool_s.tile([128, SBIG, 3], F32, tag="mk", name="mk")[:, :tslot, :]
                nc.vector.tensor_scalar(mk, wab[:, :, 3:6], 0.0, None,
                                        mybir.AluOpType.is_lt)
                nc.vector.tensor_tensor(wab[:, :, 3:6], wab[:, :, 3:6], mk, add)
                nc.vector.tensor_tensor(lf, lf, mk, sub)
                nc.vector.tensor_scalar(wab[:, :, 0:3], wab[:, :, 3:6],
                                        -1.0, 1.0, mult, add)

                # xy[dx,dy] = X_dx*Y_dy; w8[c=dx*4+dy*2+dz] = xy*Z_dz
                wv = wab.rearrange("p j (b a) -> p j b a", b=2)
                x_ap = wv[:, :, :, 0:1].to_broadcast([128, tslot, 2, 2])
                y_ap = wv[:, :, :, 1:2].transpose([0, 1, 3, 2]).to_broadcast(
                    [128, tslot, 2, 2])
                xy = pool_s.tile([128, SBIG, 4], F32, tag="xy", name="xy")[:, :tslot, :]
                nc.vector.tensor_tensor(
                    xy.rearrange("p j (a b) -> p j a b", a=2), x_ap, y_ap, mult)

                xy_b = xy.unsqueeze(3).to_broadcast([128, tslot, 4, 2])
                z_ap = wv[:, :, :, 2:3].transpose([0, 1, 3, 2]).to_broadcast(
                    [128, tslot, 4, 2])
                w8 = pool_w8.tile([128, SBIG, 8], F32, tag="w8", name="w8")[:, :tslot, :]
                nc.vector.tensor_tensor(
                    w8.rearrange("p j (a b) -> p j a b", a=4), xy_b, z_ap, mult)

                # offset = (lx - window_base_plane)*PLANE + ly*CELLS + lz
                u = pool_off.tile([128, SBIG], F32, tag="u", name="u")[:, :tslot]
                nc.vector.scalar_tensor_tensor(
                    u, lf[:, :, 1], float(CELLS), lf[:, :, 2], mult, add)
                t1 = pool_off.tile([128, SBIG], F32, tag="t1", name="t1")[:, :tslot]
                nc.vector.tensor_scalar_sub(t1, lf[:, :, 0], base_t[:, w:w + 1])
                v = pool_off.tile([128, SBIG], F32, tag="vv", name="vv")[:, :tslot]
                nc.vector.scalar_tensor_tensor(v, t1, float(PLANE), u, mult, add)
                off16 = pool_off.tile([128, SBIG], I16, tag="off16", name="off16")[:, :tslot]
                nc.vector.tensor_copy(off16, v)

                nc.sync.dma_start(
                    idx_dram[toff:toff + t_sz].rearrange("(s p) -> p s", p=128),
                    off16)
                w8s.append(w8)
                tile_offs.append(toff)
                toff += t_sz

            # Phase B: 8 group-reads -> wrap-16-replicated idx tile.
            idx_sb = pool_i.tile([128, cwmax // 16], I16, tag="idxsb", name="idxsb")[:, :cw // 16]
            wrap = idx_dram.rearrange("(c q) -> q c", q=16)
            for g8 in range(8):
                nc.sync.dma_start(idx_sb[16 * g8:16 * (g8 + 1), :], wrap)

            # Phase C: gather + weighted reduce per tile.
            for t_sz, toff, w8 in zip(sizes, tile_offs, w8s):
                tslot = t_sz // 128
                g = pool_g.tile([128, SBIG, ELEM], F32, tag="g", name="g")[:, :tslot, :]
                nc.gpsimd.dma_gather(
                    g, wslab, idx_sb[:, toff // 16:(toff + t_sz) // 16],
                    t_sz, t_sz, ELEM, single_packet=False)

                w8_b = w8.unsqueeze(3).to_broadcast([128, tslot, 8, F])
                gv = g.rearrange("p j (c f) -> p j c f", c=8)
                nc.vector.tensor_tensor(gv, gv, w8_b, mult)

                res = pool_r.tile([128, SBIG, F], F32, tag="res", name="res")[:, :tslot, :]
                nc.vector.tensor_reduce(
                    res, g.rearrange("p j (c f) -> p j f c", c=8),
                    axis=mybir.AxisListType.X, op=add)

                outv = out_d[woff + toff:woff + toff + t_sz, :].rearrange(
                    "(s p) f -> p s f", p=128)
                nc.sync.dma_start(outv, res)
            base += cw


_NC_CACHE = {}


def _get_nc(tiles_key):
    if tiles_key in _NC_CACHE:
        return _NC_CACHE[tiles_key]
    tiles_w = list(tiles_key)
    capt = sum(sum(s) for s in tiles_w)
    nc = bacc.Bacc("TRN2", target_bir_lowering=False, debug=False,
                   num_devices=NCORES)
    pts_d = nc.dram_tensor("pts", [capt, 3], F32, kind="ExternalInput").ap()
    slab_d = nc.dram_tensor("slab", [SLAB_ROWS, ELEM], F32,
                            kind="ExternalInput").ap()
    basep_d = nc.dram_tensor("basep", [128, NWIN], F32,
                             kind="ExternalInput").ap()
    out_d = nc.dram_tensor("out", [capt, F], F32, kind="ExternalOutput").ap()
    with tile.TileContext(nc) as tc:
        nc.gpsimd.load_library(mlp)
        with tc.tile_pool(name="cst", bufs=1) as pool_c, \
             tc.tile_pool(name="dram", bufs=2, space="DRAM") as pool_d:
            bt = pool_c.tile([128, NWIN], F32)
            nc.sync.dma_start(bt[:], basep_d)
            build_body(tc, out_d, pts_d, slab_d, bt, pool_d, tiles_w)
    nc.compile()
    _NC_CACHE[tiles_key] = nc
    return nc


def make_in_maps(pts, latents):
    """Host-side shard: oct-expand the grid, bucket points by x-plane pair."""
    pts = np.ascontiguousarray(np.asarray(pts, dtype=np.float32))
    latents = np.asarray(latents, dtype=np.float32)
    n = len(pts)
    lat3 = latents.reshape(R, R, R, F)

    octt = np.empty((CELLS, CELLS, CELLS, 8, F), np.float32)
    for dx in (0, 1):
        for dy in (0, 1):
            for dz in (0, 1):
                c = dx * 4 + dy * 2 + dz
                octt[:, :, :, c, :] = lat3[dx:dx + CELLS, dy:dy + CELLS,
                                           dz:dz + CELLS, :]
    octt = octt.reshape(CELLS, PLANE * ELEM)

    cell = np.floor(pts[:, 0] * np.float32(RI)).astype(np.int32)
    idx_lists = []   # [core][window] -> point index arrays
    for k in range(NCORES):
        lo, hi = BOUNDS[k], BOUNDS[k + 1]
        lists = []
        for w in range(NWIN):
            plo, phi = lo + 2 * w, lo + 2 * w + 2
            if plo >= hi:
                lists.append(np.empty(0, np.int64))
            else:
                lists.append(np.nonzero((cell >= plo) & (cell < phi))[0])
        idx_lists.append(lists)

    # tile plan per window: shared across cores (SPMD), sized to max count
    tiles_w = []
    for w in range(NWIN):
        m = max(max(len(idx_lists[k][w]) for k in range(NCORES)), 1)
        nbig, rem = divmod(m, TBIG)
        sizes = [TBIG] * nbig + ([TSMALL] * -(-rem // TSMALL) if rem else [])
        tiles_w.append(tuple(sizes))
    tiles_key = tuple(tiles_w)
    capt = sum(sum(s) for s in tiles_w)

    in_maps, posmaps = [], []
    for k in range(NCORES):
        lo = BOUNDS[k]
        p = np.empty((capt, 3), np.float32)
        pos = np.full(capt, -1, np.int64)
        off = 0
        for w in range(NWIN):
            ids = idx_lists[k][w]
            cw = sum(tiles_w[w])
            p[off:off + len(ids)] = pts[ids]
            # dummy pad point inside this window's first plane
            p[off + len(ids):off + cw] = np.float32((lo + 2 * w + 0.5) / RI)
            pos[off:off + len(ids)] = ids
            off += cw
        if BOUNDS[k + 1] - lo == MAXPLANES:
            slab = octt[lo:BOUNDS[k + 1]].reshape(SLAB_ROWS, ELEM)
        else:
            slab = np.zeros((SLAB_ROWS, ELEM), np.float32)
            slab[:(BOUNDS[k + 1] - lo) * PLANE] = \
                octt[lo:BOUNDS[k + 1]].reshape(-1, ELEM)
        basep = np.tile(
            np.array([lo + 2 * w for w in range(NWIN)], np.float32), (128, 1))
        in_maps.append({"pts": p, "slab": slab, "basep": basep})
        posmaps.append(pos)
    return in_maps, posmaps, tiles_key, n


def kernel(pts, latents, _trace=False):
    in_maps, posmaps, tiles_key, n = make_in_maps(pts, latents)
    nc = _get_nc(tiles_key)
    r = run_bass_kernel_spmd(nc, in_maps, core_ids=list(range(NCORES)),
                             trace=_trace)
    out = np.empty((n, F), np.float32)
    for k in range(NCORES):
        pos = posmaps[k]
        m = pos >= 0
        out[pos[m]] = r.results[k]["out"][m]
    if _trace:
        kernel.last_exec_time_ns = r.exec_time_ns
        kernel.last_results = r
    return out


# revision 11
# speedup vs baseline: 2.6130x; 2.6130x over previous
"""Trilinear grid-encoding lookup (embedding_lookup) on 8 trn2 NeuronCores.

Strategy:
  - Host: expand the 128^3 x 16 latent grid into an "oct" table where entry
    (x, y, z) holds all 8 corner feature rows of cell (x,y,z) contiguously
    (8*16 f32 = 512 B).  Shard points by their x-cell into 8 contiguous
    lx-ranges (16 planes each); each core gets its oct-table slab (~130 MB)
    plus its point list, counting-sorted into 8 two-plane "windows" so that
    every gather stays inside a 31752-row window (int16 index range).
    Points are pre-permuted on the host so every device DMA is dense.
  - Device (per core): Vector engine computes cell indices + fractions + the
    8 trilinear weights (pass 1, gather-layout) and window-relative int16
    gather indices (pass 2, wrap-source layout); the indices take a DRAM
    round-trip (8 dense writes + 2 dense reads) to reach the wrap-16 layout
    dma_gather's Q7 cores 0/1 consume; dma_gather pulls one 512-B oct entry
    per point; Vector engine multiplies by broadcast weights and reduces
    over the 8 corners.
  - Host: inverse-permute the 8 core outputs into the full [N, 16] result.
"""

import numpy as np

import concourse.bass as bass
import concourse.tile as tile
from concourse import bacc, mybir
from concourse.bass_utils import run_bass_kernel_spmd
from concourse.library_config import mlp

F32 = mybir.dt.float32
I16 = mybir.dt.int16
I32 = mybir.dt.int32

# Problem constants (hardcoded per harness contract).
R = 128                  # grid resolution
RI = 126                 # R - 2 (coordinate scale)
F = 16                   # feature dim
NCORES = 8

# Sharding constants.
CELLS = 126              # lx/ly/lz take values 0..125
PLANE = CELLS * CELLS    # oct entries per x-plane (15876)
BOUNDS = [0, 16, 32, 48, 64, 80, 96, 112, 126]   # lx plane ranges per core
MAXPLANES = 16
NWIN = 8                 # two-plane gather windows per core
WROWS = 2 * PLANE        # oct rows per window (31752 < 2^15)
SLAB_ROWS = MAXPLANES * PLANE
ELEM = 8 * F             # f32 per oct entry (512 B)

TBIG = 8192              # points per big tile
SBIG = TBIG // 128       # slots per partition in a big tile


def build_body(tc, out_d, pts_d, pts2_d, slab_d, base_t, pool_d, tiles_w):
    """Per-core tile kernel.

    tiles_w: per-window tuple of tile sizes (multiples of 128, <= TBIG).
    pts: pass-1 point order (gather layout), pts2: pass-2 order (wrap
    source layout); out in pass-1 order.  base_t: [128, NWIN] f32 window
    base planes (per core).
    """
    nc = tc.nc
    mult = mybir.AluOpType.mult
    add = mybir.AluOpType.add
    sub = mybir.AluOpType.subtract

    cwmax = max(sum(s) for s in tiles_w)

    with (
        tc.tile_pool(name="small", bufs=3) as pool_s,
        tc.tile_pool(name="p2", bufs=2) as pool_2,
        tc.tile_pool(name="w8p", bufs=12) as pool_w8,
        tc.tile_pool(name="g", bufs=2) as pool_g,
        tc.tile_pool(name="idxs", bufs=2) as pool_i,
        tc.tile_pool(name="res", bufs=3) as pool_r,
    ):
        base = 0                    # running point offset in pts/pts2/out
        for w, sizes in enumerate(tiles_w):
            cw = sum(sizes)
            woff = base
            wslab = slab_d[w * WROWS:(w + 1) * WROWS, :]
            C = cw // 16            # wrap columns
            Cs = cw // 128          # wrap columns per source partition-block

            # ---- Pass 2: window-relative offsets in wrap-source layout.
            p2v = pts2_d[woff:woff + cw, :].rearrange("(p c) x -> p c x", p=128)
            p2_sb = pool_2.tile([128, cwmax // 128, 3], F32, tag="p2pts",
                                name="p2pts")[:, :Cs, :]
            nc.sync.dma_start(p2_sb, p2v)
            s2 = pool_2.tile([128, cwmax // 128, 3], F32, tag="s2",
                             name="s2")[:, :Cs, :]
            nc.vector.tensor_scalar_mul(s2, p2_sb, float(RI))
            li2 = pool_2.tile([128, cwmax // 128, 3], I32, tag="li2",
                              name="li2")[:, :Cs, :]
            nc.vector.tensor_copy(li2, s2)
            lf2 = pool_2.tile([128, cwmax // 128, 3], F32, tag="lf2",
                              name="lf2")[:, :Cs, :]
            nc.vector.tensor_copy(lf2, li2)
            fb2 = pool_2.tile([128, cwmax // 128, 3], F32, tag="fb2",
                              name="fb2")[:, :Cs, :]
            nc.vector.tensor_tensor(fb2, s2, lf2, sub)
            mk2 = pool_2.tile([128, cwmax // 128, 3], F32, tag="mk2",
                              name="mk2")[:, :Cs, :]
            nc.vector.tensor_scalar(mk2, fb2, 0.0, None, mybir.AluOpType.is_lt)
            nc.vector.tensor_tensor(lf2, lf2, mk2, sub)  # lf2 = floor(s2)
            u2 = pool_2.tile([128, cwmax // 128], F32, tag="u2",
                             name="u2")[:, :Cs]
            nc.vector.scalar_tensor_tensor(
                u2, lf2[:, :, 1], float(CELLS), lf2[:, :, 2], mult, add)
            t2 = pool_2.tile([128, cwmax // 128], F32, tag="t2",
                             name="t2")[:, :Cs]
            nc.vector.tensor_scalar_sub(t2, lf2[:, :, 0], base_t[:, w:w + 1])
            v2 = pool_2.tile([128, cwmax // 128], F32, tag="v2",
                             name="v2")[:, :Cs]
            nc.vector.scalar_tensor_tensor(v2, t2, float(PLANE), u2, mult, add)
            off16 = pool_2.tile([128, cwmax // 128], I16, tag="off16",
                                name="off16")[:, :Cs]
            nc.vector.tensor_copy(off16, v2)

            # 8 dense writes: partitions [16a,16a+16) -> rows b, cols a*Cs..
            idx_dram = pool_d.tile([cwmax], I16, tag="idxd", name="idxd")[:cw]
            idv = idx_dram.rearrange("(b c) -> b c", b=16)
            for a in range(8):
                nc.sync.dma_start(idv[:, a * Cs:(a + 1) * Cs],
                                  off16[16 * a:16 * (a + 1), :])
            # 2 dense reads into idx groups 0/1 (Q7 cores 0/1 read these);
            # zero the rest on the otherwise-idle Scalar engine (the sim
            # bounds-checks all 128 partitions).
            idx_sb = pool_i.tile([128, cwmax // 16], I16, tag="idxsb",
                                 name="idxsb")[:, :C]
            nc.scalar.memzero(idx_sb)
            for g8 in range(2):
                nc.sync.dma_start(idx_sb[16 * g8:16 * (g8 + 1), :], idv)

            # ---- Pass 1 (per tile): trilinear weights in gather layout.
            w8s, tile_offs = [], []
            toff = 0
            for t_sz in sizes:
                tslot = t_sz // 128
                ptsv = pts_d[woff + toff:woff + toff + t_sz, :].rearrange(
                    "(p s) c -> p s c", p=128)
                pts_sb = pool_s.tile([128, SBIG, 3], F32, tag="pts",
                                     name="pts")[:, :tslot, :]
                nc.sync.dma_start(pts_sb, ptsv)

                s = pool_s.tile([128, SBIG, 3], F32, tag="s", name="s")[:, :tslot, :]
                nc.vector.tensor_scalar_mul(s, pts_sb, float(RI))
                li = pool_s.tile([128, SBIG, 3], I32, tag="li", name="li")[:, :tslot, :]
                nc.vector.tensor_copy(li, s)
                lf = pool_s.tile([128, SBIG, 3], F32, tag="lf", name="lf")[:, :tslot, :]
                nc.vector.tensor_copy(lf, li)

                # wab[:, :, 0:3] = 1 - frac, wab[:, :, 3:6] = frac in [0,1)
                wab = pool_s.tile([128, SBIG, 6], F32, tag="wab", name="wab")[:, :tslot, :]
                nc.vector.tensor_tensor(wab[:, :, 3:6], s, lf, sub)
                mk = pool_s.tile([128, SBIG, 3], F32, tag="mk", name="mk")[:, :tslot, :]
                nc.vector.tensor_scalar(mk, wab[:, :, 3:6], 0.0, None,
                                        mybir.AluOpType.is_lt)
                nc.vector.tensor_tensor(wab[:, :, 3:6], wab[:, :, 3:6], mk, add)
                nc.vector.tensor_scalar(wab[:, :, 0:3], wab[:, :, 3:6],
                                        -1.0, 1.0, mult, add)

                # xy[dx,dy] = X_dx*Y_dy; w8[c=dx*4+dy*2+dz] = xy*Z_dz
                wv = wab.rearrange("p j (b a) -> p j b a", b=2)
                x_ap = wv[:, :, :, 0:1].to_broadcast([128, tslot, 2, 2])
                y_ap = wv[:, :, :, 1:2].transpose([0, 1, 3, 2]).to_broadcast(
                    [128, tslot, 2, 2])
                xy = pool_s.tile([128, SBIG, 4], F32, tag="xy", name="xy")[:, :tslot, :]
                nc.vector.tensor_tensor(
                    xy.rearrange("p j (a b) -> p j a b", a=2), x_ap, y_ap, mult)

                xy_b = xy.unsqueeze(3).to_broadcast([128, tslot, 4, 2])
                z_ap = wv[:, :, :, 2:3].transpose([0, 1, 3, 2]).to_broadcast(
                    [128, tslot, 4, 2])
                w8 = pool_w8.tile([128, SBIG, 8], F32, tag="w8", name="w8")[:, :tslot, :]
                nc.vector.tensor_tensor(
                    w8.rearrange("p j (a b) -> p j a b", a=4), xy_b, z_ap, mult)
                w8s.append(w8)
                tile_offs.append(toff)
                toff += t_sz

            # ---- Phase C: gather + weighted reduce per tile.
            for t_sz, toff, w8 in zip(sizes, tile_offs, w8s):
                tslot = t_sz // 128
                g = pool_g.tile([128, SBIG, ELEM], F32, tag="g",
                                name="g")[:, :tslot, :]
                nc.gpsimd.dma_gather(
                    g, wslab, idx_sb[:, toff // 16:(toff + t_sz) // 16],
                    t_sz, t_sz, ELEM, single_packet=False)

                w8_b = w8.unsqueeze(3).to_broadcast([128, tslot, 8, F])
                gv = g.rearrange("p j (c f) -> p j c f", c=8)
                nc.vector.tensor_tensor(gv, gv, w8_b, mult)

                res = pool_r.tile([128, SBIG, F], F32, tag="res",
                                  name="res")[:, :tslot, :]
                nc.vector.tensor_reduce(
                    res, g.rearrange("p j (c f) -> p j f c", c=8),
                    axis=mybir.AxisListType.X, op=add)

                outv = out_d[woff + toff:woff + toff + t_sz, :].rearrange(
                    "(p s) f -> p s f", p=128)
                nc.sync.dma_start(outv, res)
            base += cw


_NC_CACHE = {}


def _get_nc(tiles_key):
    if tiles_key in _NC_CACHE:
        return _NC_CACHE[tiles_key]
    tiles_w = list(tiles_key)
    capt = sum(sum(s) for s in tiles_w)
    nc = bacc.Bacc("TRN2", target_bir_lowering=False, debug=False,
                   num_devices=NCORES)
    pts_d = nc.dram_tensor("pts", [capt, 3], F32, kind="ExternalInput").ap()
    pts2_d = nc.dram_tensor("pts2", [capt, 3], F32, kind="ExternalInput").ap()
    slab_d = nc.dram_tensor("slab", [SLAB_ROWS, ELEM], F32,
                            kind="ExternalInput").ap()
    basep_d = nc.dram_tensor("basep", [128, NWIN], F32,
                             kind="ExternalInput").ap()
    out_d = nc.dram_tensor("out", [capt, F], F32, kind="ExternalOutput").ap()
    with tile.TileContext(nc) as tc:
        nc.gpsimd.load_library(mlp)
        with tc.tile_pool(name="cst", bufs=1) as pool_c, \
             tc.tile_pool(name="dram", bufs=2, space="DRAM") as pool_d:
            bt = pool_c.tile([128, NWIN], F32, name="bt")
            nc.sync.dma_start(bt[:], basep_d)
            build_body(tc, out_d, pts_d, pts2_d, slab_d, bt, pool_d, tiles_w)
    nc.compile()
    _NC_CACHE[tiles_key] = nc
    return nc


def _pass1_perm(t_sz):
    """Within-tile host row r = p*tslot + s holds gather-index i = s*128 + p."""
    tslot = t_sz // 128
    i = np.arange(t_sz).reshape(tslot, 128)      # [s, p] -> i
    return i.T.reshape(-1)                       # row r = p*tslot+s -> i


def _pass2_perm(cw):
    """Within-window host row r2 = p2*Cs + c2 holds i = ((p2//16)*Cs+c2)*16+p2%16."""
    Cs = cw // 128
    p2 = np.arange(128)[:, None]
    c2 = np.arange(Cs)[None, :]
    i = ((p2 // 16) * Cs + c2) * 16 + (p2 % 16)
    return i.reshape(-1)


def make_in_maps(pts, latents):
    """Host-side shard: oct-expand the grid, bucket points by x-plane pair."""
    pts = np.ascontiguousarray(np.asarray(pts, dtype=np.float32))
    latents = np.asarray(latents, dtype=np.float32)
    n = len(pts)
    lat3 = latents.reshape(R, R, R, F)

    octt = np.empty((CELLS, CELLS, CELLS, 8, F), np.float32)
    for dx in (0, 1):
        for dy in (0, 1):
            for dz in (0, 1):
                c = dx * 4 + dy * 2 + dz
                octt[:, :, :, c, :] = lat3[dx:dx + CELLS, dy:dy + CELLS,
                                           dz:dz + CELLS, :]
    octt = octt.reshape(CELLS, PLANE * ELEM)

    cell = np.floor(pts[:, 0] * np.float32(RI)).astype(np.int32)
    idx_lists = []   # [core][window] -> point index arrays
    for k in range(NCORES):
        lo, hi = BOUNDS[k], BOUNDS[k + 1]
        lists = []
        for w in range(NWIN):
            plo = lo + 2 * w
            if plo >= hi:
                lists.append(np.empty(0, np.int64))
            else:
                lists.append(np.nonzero((cell >= plo) & (cell < plo + 2))[0])
        idx_lists.append(lists)

    # tile plan per window: shared across cores (SPMD), sized to max count
    tiles_w = []
    for w in range(NWIN):
        m = max(max(len(idx_lists[k][w]) for k in range(NCORES)), 1)
        m = -(-m // 128) * 128
        nbig, rem = divmod(m, TBIG)
        sizes = [TBIG] * nbig + ([rem] if rem else [])
        tiles_w.append(tuple(sizes))
    tiles_key = tuple(tiles_w)
    capt = sum(sum(s) for s in tiles_w)

    # per-window permutations (same for all cores)
    perms1, perms2 = [], []
    for w in range(NWIN):
        cw = sum(tiles_w[w])
        pw = np.empty(cw, np.int64)
        toff = 0
        for t_sz in tiles_w[w]:
            pw[toff:toff + t_sz] = toff + _pass1_perm(t_sz)
            toff += t_sz
        perms1.append(pw)
        perms2.append(_pass2_perm(cw))

    in_maps, posmaps = [], []
    for k in range(NCORES):
        lo = BOUNDS[k]
        p1 = np.empty((capt, 3), np.float32)
        p2 = np.empty((capt, 3), np.float32)
        pos = np.full(capt, -1, np.int64)
        off = 0
        for w in range(NWIN):
            ids = idx_lists[k][w]
            cw = sum(tiles_w[w])
            # window-local point array in gather-i order (pads at the end)
            wpts = np.empty((cw, 3), np.float32)
            wpts[:len(ids)] = pts[ids]
            wpts[len(ids):] = np.float32((lo + 2 * w + 0.5) / RI)
            wpos = np.full(cw, -1, np.int64)
            wpos[:len(ids)] = ids
            p1[off:off + cw] = wpts[perms1[w]]
            p2[off:off + cw] = wpts[perms2[w]]
            pos[off:off + cw] = wpos[perms1[w]]
            off += cw
        if BOUNDS[k + 1] - lo == MAXPLANES:
            slab = octt[lo:BOUNDS[k + 1]].reshape(SLAB_ROWS, ELEM)
        else:
            slab = np.zeros((SLAB_ROWS, ELEM), np.float32)
            slab[:(BOUNDS[k + 1] - lo) * PLANE] = \
                octt[lo:BOUNDS[k + 1]].reshape(-1, ELEM)
        basep = np.tile(
            np.array([lo + 2 * w for w in range(NWIN)], np.float32), (128, 1))
        in_maps.append({"pts": p1, "pts2": p2, "slab": slab, "basep": basep})
        posmaps.append(pos)
    return in_maps, posmaps, tiles_key, n


def kernel(pts, latents, _trace=False):
    in_maps, posmaps, tiles_key, n = make_in_maps(pts, latents)
    nc = _get_nc(tiles_key)
    r = run_bass_kernel_spmd(nc, in_maps, core_ids=list(range(NCORES)),
                             trace=_trace)
    out = np.empty((n, F), np.float32)
    for k in range(NCORES):
        pos = posmaps[k]
        m = pos >= 0
        out[pos[m]] = r.results[k]["out"][m]
    if _trace:
        kernel.last_exec_time_ns = r.exec_time_ns
        kernel.last_results = r
    return out


# revision 12
# speedup vs baseline: 2.6949x; 1.0313x over previous
"""Trilinear grid-encoding lookup (embedding_lookup) on 8 trn2 NeuronCores.

Strategy:
  - Host: expand the 128^3 x 16 latent grid into an "oct" table where entry
    (x, y, z) holds all 8 corner feature rows of cell (x,y,z) contiguously
    (8*16 f32 = 512 B).  Shard points by their x-cell into 8 contiguous
    lx-ranges (16 planes each); each core gets its oct-table slab (~130 MB)
    plus its point list, counting-sorted into 8 two-plane "windows" so that
    every gather stays inside a 31752-row window (int16 index range).
    Points are pre-permuted on the host so every device DMA is dense.
  - Device (per core): Vector engine computes cell indices + fractions + the
    8 trilinear weights (pass 1, gather-layout) and window-relative int16
    gather indices (pass 2, wrap-source layout); the indices take a DRAM
    round-trip (8 dense writes + 2 dense reads) to reach the wrap-16 layout
    dma_gather's Q7 cores 0/1 consume; dma_gather pulls one 512-B oct entry
    per point; Vector engine multiplies by broadcast weights and reduces
    over the 8 corners.
  - Host: inverse-permute the 8 core outputs into the full [N, 16] result.
"""

import numpy as np

import concourse.bass as bass
import concourse.tile as tile
from concourse import bacc, mybir
from concourse.bass_utils import run_bass_kernel_spmd
from concourse.library_config import mlp

F32 = mybir.dt.float32
I16 = mybir.dt.int16
I32 = mybir.dt.int32

# Problem constants (hardcoded per harness contract).
R = 128                  # grid resolution
RI = 126                 # R - 2 (coordinate scale)
F = 16                   # feature dim
NCORES = 8

# Sharding constants.
CELLS = 126              # lx/ly/lz take values 0..125
PLANE = CELLS * CELLS    # oct entries per x-plane (15876)
BOUNDS = [0, 16, 32, 48, 64, 80, 96, 112, 126]   # lx plane ranges per core
MAXPLANES = 16
NWIN = 8                 # two-plane gather windows per core
WROWS = 2 * PLANE        # oct rows per window (31752 < 2^15)
SLAB_ROWS = MAXPLANES * PLANE
ELEM = 8 * F             # f32 per oct entry (512 B)

TBIG = 8192              # points per big tile
SBIG = TBIG // 128       # slots per partition in a big tile


def build_body(tc, out_d, pts_d, pts2_d, slab_d, base_t, pool_d, tiles_w):
    """Per-core tile kernel.

    tiles_w: per-window tuple of tile sizes (multiples of 128, <= TBIG).
    pts: pass-1 point order (gather layout), pts2: pass-2 order (wrap
    source layout); out in pass-1 order.  base_t: [128, NWIN] f32 window
    base planes (per core).
    """
    nc = tc.nc
    mult = mybir.AluOpType.mult
    add = mybir.AluOpType.add
    sub = mybir.AluOpType.subtract

    cwmax = max(sum(s) for s in tiles_w)

    with (
        tc.tile_pool(name="small", bufs=3) as pool_s,
        tc.tile_pool(name="p2", bufs=2) as pool_2,
        tc.tile_pool(name="w8p", bufs=12) as pool_w8,
        tc.tile_pool(name="g", bufs=2) as pool_g,
        tc.tile_pool(name="idxs", bufs=4) as pool_i,
        tc.tile_pool(name="res", bufs=2) as pool_r,
    ):
        # ---- Phase A (all windows): offsets -> DRAM -> wrap idx tiles,
        # so the GpSimd gather stream can free-run afterwards.
        idx_sbs = []
        base = 0
        for w, sizes in enumerate(tiles_w):
            cw = sum(sizes)
            woff = base
            C = cw // 16            # wrap columns
            Cs = cw // 128          # wrap columns per source partition-block

            p2v = pts2_d[woff:woff + cw, :].rearrange("(p c) x -> p c x", p=128)
            p2_sb = pool_2.tile([128, cwmax // 128, 3], F32, tag="p2pts",
                                name="p2pts")[:, :Cs, :]
            nc.sync.dma_start(p2_sb, p2v)
            s2 = pool_2.tile([128, cwmax // 128, 3], F32, tag="s2",
                             name="s2")[:, :Cs, :]
            nc.vector.tensor_scalar_mul(s2, p2_sb, float(RI))
            li2 = pool_2.tile([128, cwmax // 128, 3], I32, tag="li2",
                              name="li2")[:, :Cs, :]
            nc.vector.tensor_copy(li2, s2)
            lf2 = pool_2.tile([128, cwmax // 128, 3], F32, tag="lf2",
                              name="lf2")[:, :Cs, :]
            nc.vector.tensor_copy(lf2, li2)
            fb2 = pool_2.tile([128, cwmax // 128, 3], F32, tag="fb2",
                              name="fb2")[:, :Cs, :]
            nc.vector.tensor_tensor(fb2, s2, lf2, sub)
            mk2 = pool_2.tile([128, cwmax // 128, 3], F32, tag="mk2",
                              name="mk2")[:, :Cs, :]
            nc.vector.tensor_scalar(mk2, fb2, 0.0, None, mybir.AluOpType.is_lt)
            nc.vector.tensor_tensor(lf2, lf2, mk2, sub)  # lf2 = floor(s2)
            u2 = pool_2.tile([128, cwmax // 128], F32, tag="u2",
                             name="u2")[:, :Cs]
            nc.vector.scalar_tensor_tensor(
                u2, lf2[:, :, 1], float(CELLS), lf2[:, :, 2], mult, add)
            t2 = pool_2.tile([128, cwmax // 128], F32, tag="t2",
                             name="t2")[:, :Cs]
            nc.vector.tensor_scalar_sub(t2, lf2[:, :, 0], base_t[:, w:w + 1])
            v2 = pool_2.tile([128, cwmax // 128], F32, tag="v2",
                             name="v2")[:, :Cs]
            nc.vector.scalar_tensor_tensor(v2, t2, float(PLANE), u2, mult, add)
            off16 = pool_2.tile([128, cwmax // 128], I16, tag="off16",
                                name="off16")[:, :Cs]
            nc.vector.tensor_copy(off16, v2)

            # 8 dense writes: partitions [16a,16a+16) -> rows b, cols a*Cs..
            idx_dram = pool_d.tile([cwmax], I16, tag="idxd", name="idxd")[:cw]
            idv = idx_dram.rearrange("(b c) -> b c", b=16)
            for a in range(8):
                nc.sync.dma_start(idv[:, a * Cs:(a + 1) * Cs],
                                  off16[16 * a:16 * (a + 1), :])
            # 2 dense reads into idx groups 0/1 (Q7 cores 0/1 read these);
            # zero the rest on the otherwise-idle Scalar engine (the sim
            # bounds-checks all 128 partitions).
            idx_sb = pool_i.tile([128, cwmax // 16], I16, tag="idxsb",
                                 name="idxsb")[:, :C]
            nc.scalar.memzero(idx_sb)
            for g8 in range(2):
                nc.sync.dma_start(idx_sb[16 * g8:16 * (g8 + 1), :], idv)
            idx_sbs.append(idx_sb)
            base += cw

        # ---- Per tile: weights (pass 1) + gather + weighted reduce.
        base = 0
        for w, sizes in enumerate(tiles_w):
            cw = sum(sizes)
            woff = base
            wslab = slab_d[w * WROWS:(w + 1) * WROWS, :]
            idx_sb = idx_sbs[w]
            toff = 0
            for t_sz in sizes:
                tslot = t_sz // 128
                ptsv = pts_d[woff + toff:woff + toff + t_sz, :].rearrange(
                    "(p s) c -> p s c", p=128)
                pts_sb = pool_s.tile([128, SBIG, 3], F32, tag="pts",
                                     name="pts")[:, :tslot, :]
                nc.sync.dma_start(pts_sb, ptsv)

                s = pool_s.tile([128, SBIG, 3], F32, tag="s", name="s")[:, :tslot, :]
                nc.vector.tensor_scalar_mul(s, pts_sb, float(RI))
                li = pool_s.tile([128, SBIG, 3], I32, tag="li", name="li")[:, :tslot, :]
                nc.vector.tensor_copy(li, s)
                lf = pool_s.tile([128, SBIG, 3], F32, tag="lf", name="lf")[:, :tslot, :]
                nc.vector.tensor_copy(lf, li)

                # wab[:, :, 0:3] = 1 - frac, wab[:, :, 3:6] = frac in [0,1)
                wab = pool_s.tile([128, SBIG, 6], F32, tag="wab", name="wab")[:, :tslot, :]
                nc.vector.tensor_tensor(wab[:, :, 3:6], s, lf, sub)
                mk = pool_s.tile([128, SBIG, 3], F32, tag="mk", name="mk")[:, :tslot, :]
                nc.vector.tensor_scalar(mk, wab[:, :, 3:6], 0.0, None,
                                        mybir.AluOpType.is_lt)
                nc.vector.tensor_tensor(wab[:, :, 3:6], wab[:, :, 3:6], mk, add)
                nc.vector.tensor_scalar(wab[:, :, 0:3], wab[:, :, 3:6],
                                        -1.0, 1.0, mult, add)

                # xy[dx,dy] = X_dx*Y_dy; w8[c=dx*4+dy*2+dz] = xy*Z_dz
                wv = wab.rearrange("p j (b a) -> p j b a", b=2)
                x_ap = wv[:, :, :, 0:1].to_broadcast([128, tslot, 2, 2])
                y_ap = wv[:, :, :, 1:2].transpose([0, 1, 3, 2]).to_broadcast(
                    [128, tslot, 2, 2])
                xy = pool_s.tile([128, SBIG, 4], F32, tag="xy", name="xy")[:, :tslot, :]
                nc.vector.tensor_tensor(
                    xy.rearrange("p j (a b) -> p j a b", a=2), x_ap, y_ap, mult)

                xy_b = xy.unsqueeze(3).to_broadcast([128, tslot, 4, 2])
                z_ap = wv[:, :, :, 2:3].transpose([0, 1, 3, 2]).to_broadcast(
                    [128, tslot, 4, 2])
                w8 = pool_w8.tile([128, SBIG, 8], F32, tag="w8", name="w8")[:, :tslot, :]
                nc.vector.tensor_tensor(
                    w8.rearrange("p j (a b) -> p j a b", a=4), xy_b, z_ap, mult)

                g = pool_g.tile([128, SBIG, ELEM], F32, tag="g",
                                name="g")[:, :tslot, :]
                nc.gpsimd.dma_gather(
                    g, wslab, idx_sb[:, toff // 16:(toff + t_sz) // 16],
                    t_sz, t_sz, ELEM, single_packet=False)

                w8_b = w8.unsqueeze(3).to_broadcast([128, tslot, 8, F])
                gv = g.rearrange("p j (c f) -> p j c f", c=8)
                nc.vector.tensor_tensor(gv, gv, w8_b, mult)

                res = pool_r.tile([128, SBIG, F], F32, tag="res",
                                  name="res")[:, :tslot, :]
                nc.vector.tensor_reduce(
                    res, g.rearrange("p j (c f) -> p j f c", c=8),
                    axis=mybir.AxisListType.X, op=add)

                outv = out_d[woff + toff:woff + toff + t_sz, :].rearrange(
                    "(p s) f -> p s f", p=128)
                nc.sync.dma_start(outv, res)
                toff += t_sz
            base += cw


_NC_CACHE = {}


def _get_nc(tiles_key):
    if tiles_key in _NC_CACHE:
        return _NC_CACHE[tiles_key]
    tiles_w = list(tiles_key)
    capt = sum(sum(s) for s in tiles_w)
    nc = bacc.Bacc("TRN2", target_bir_lowering=False, debug=False,
                   num_devices=NCORES)
    pts_d = nc.dram_tensor("pts", [capt, 3], F32, kind="ExternalInput").ap()
    pts2_d = nc.dram_tensor("pts2", [capt, 3], F32, kind="ExternalInput").ap()
    slab_d = nc.dram_tensor("slab", [SLAB_ROWS, ELEM], F32,
                            kind="ExternalInput").ap()
    basep_d = nc.dram_tensor("basep", [128, NWIN], F32,
                             kind="ExternalInput").ap()
    out_d = nc.dram_tensor("out", [capt, F], F32, kind="ExternalOutput").ap()
    with tile.TileContext(nc) as tc:
        nc.gpsimd.load_library(mlp)
        with tc.tile_pool(name="cst", bufs=1) as pool_c, \
             tc.tile_pool(name="dram", bufs=2, space="DRAM") as pool_d:
            bt = pool_c.tile([128, NWIN], F32, name="bt")
            nc.sync.dma_start(bt[:], basep_d)
            build_body(tc, out_d, pts_d, pts2_d, slab_d, bt, pool_d, tiles_w)
    nc.compile()
    _NC_CACHE[tiles_key] = nc
    return nc


def _pass1_perm(t_sz):
    """Within-tile host row r = p*tslot + s holds gather-index i = s*128 + p."""
    tslot = t_sz // 128
    i = np.arange(t_sz).reshape(tslot, 128)      # [s, p] -> i
    return i.T.reshape(-1)                       # row r = p*tslot+s -> i


def _pass2_perm(cw):
    """Within-window host row r2 = p2*Cs + c2 holds i = ((p2//16)*Cs+c2)*16+p2%16."""
    Cs = cw // 128
    p2 = np.arange(128)[:, None]
    c2 = np.arange(Cs)[None, :]
    i = ((p2 // 16) * Cs + c2) * 16 + (p2 % 16)
    return i.reshape(-1)


def make_in_maps(pts, latents):
    """Host-side shard: oct-expand the grid, bucket points by x-plane pair."""
    pts = np.ascontiguousarray(np.asarray(pts, dtype=np.float32))
    latents = np.asarray(latents, dtype=np.float32)
    n = len(pts)
    lat3 = latents.reshape(R, R, R, F)

    octt = np.empty((CELLS, CELLS, CELLS, 8, F), np.float32)
    for dx in (0, 1):
        for dy in (0, 1):
            for dz in (0, 1):
                c = dx * 4 + dy * 2 + dz
                octt[:, :, :, c, :] = lat3[dx:dx + CELLS, dy:dy + CELLS,
                                           dz:dz + CELLS, :]
    octt = octt.reshape(CELLS, PLANE * ELEM)

    cell = np.floor(pts[:, 0] * np.float32(RI)).astype(np.int32)
    idx_lists = []   # [core][window] -> point index arrays
    for k in range(NCORES):
        lo, hi = BOUNDS[k], BOUNDS[k + 1]
        lists = []
        for w in range(NWIN):
            plo = lo + 2 * w
            if plo >= hi:
                lists.append(np.empty(0, np.int64))
            else:
                lists.append(np.nonzero((cell >= plo) & (cell < plo + 2))[0])
        idx_lists.append(lists)

    # tile plan per window: shared across cores (SPMD), sized to max count
    tiles_w = []
    for w in range(NWIN):
        m = max(max(len(idx_lists[k][w]) for k in range(NCORES)), 1)
        m = -(-m // 128) * 128
        nbig, rem = divmod(m, TBIG)
        sizes = [TBIG] * nbig + ([rem] if rem else [])
        tiles_w.append(tuple(sizes))
    tiles_key = tuple(tiles_w)
    capt = sum(sum(s) for s in tiles_w)

    # per-window permutations (same for all cores)
    perms1, perms2 = [], []
    for w in range(NWIN):
        cw = sum(tiles_w[w])
        pw = np.empty(cw, np.int64)
        toff = 0
        for t_sz in tiles_w[w]:
            pw[toff:toff + t_sz] = toff + _pass1_perm(t_sz)
            toff += t_sz
        perms1.append(pw)
        perms2.append(_pass2_perm(cw))

    in_maps, posmaps = [], []
    for k in range(NCORES):
        lo = BOUNDS[k]
        p1 = np.empty((capt, 3), np.float32)
        p2 = np.empty((capt, 3), np.float32)
        pos = np.full(capt, -1, np.int64)
        off = 0
        for w in range(NWIN):
            ids = idx_lists[k][w]
            cw = sum(tiles_w[w])
            # window-local point array in gather-i order (pads at the end)
            wpts = np.empty((cw, 3), np.float32)
            wpts[:len(ids)] = pts[ids]
            wpts[len(ids):] = np.float32((lo + 2 * w + 0.5) / RI)
            wpos = np.full(cw, -1, np.int64)
            wpos[:len(ids)] = ids
            p1[off:off + cw] = wpts[perms1[w]]
            p2[off:off + cw] = wpts[perms2[w]]
            pos[off:off + cw] = wpos[perms1[w]]
            off += cw
        if BOUNDS[k + 1] - lo == MAXPLANES:
            slab = octt[lo:BOUNDS[k + 1]].reshape(SLAB_ROWS, ELEM)
        else:
            slab = np.zeros((SLAB_ROWS, ELEM), np.float32)
            slab[:(BOUNDS[k + 1] - lo) * PLANE] = \
                octt[lo:BOUNDS[k + 1]].reshape(-1, ELEM)
        basep = np.tile(
            np.array([lo + 2 * w for w in range(NWIN)], np.float32), (128, 1))
        in_maps.append({"pts": p1, "pts2": p2, "slab": slab, "basep": basep})
        posmaps.append(pos)
    return in_maps, posmaps, tiles_key, n


def kernel(pts, latents, _trace=False):
    in_maps, posmaps, tiles_key, n = make_in_maps(pts, latents)
    nc = _get_nc(tiles_key)
    r = run_bass_kernel_spmd(nc, in_maps, core_ids=list(range(NCORES)),
                             trace=_trace)
    out = np.empty((n, F), np.float32)
    for k in range(NCORES):
        pos = posmaps[k]
        m = pos >= 0
        out[pos[m]] = r.results[k]["out"][m]
    if _trace:
        kernel.last_exec_time_ns = r.exec_time_ns
        kernel.last_results = r
    return out
